# revision 8
# baseline (speedup 1.0000x reference)
"""Diagonal SSM kernel for Trainium2 (8 NeuronCores, batch-parallel).

Computes, for x [8, 4096, 1024], W_decay/W_input [1024, 1024], biases [1024]:
    decays     = sigmoid(x @ W_decay.T + b_decay)
    injections = x @ W_input.T + b_input
    states_t   = decays_t * states_{t-1} + injections_t      (scan over T)

Sharding: batch b -> core b (8 batches, 8 cores, no collectives).

Device-side layout is chosen so the tensor engine does ONLY the 1024
bf16 matmuls (213 ns each, ~218 us total) with zero transposes:
  - host feeds x^T [d, t] pre-cast bf16 per core, so the contraction dim
    d is already on partitions,
  - host feeds W_decay^T / W_input^T [d, e] pre-cast bf16 (stationary
    operands sliced per 128-channel block),
  - both projections accumulate fp32 in PSUM (8 banks rotating),
  - sigmoid(z + b_decay) / (z + b_input) on the scalar engine straight
    out of PSUM,
  - the recurrence is a native DVE tensor_tensor_scan per [128 x 512]
    tile (fp32 state), chained across panels through its `initial`
    operand,
  - states stored as y^T [d, t] straight from SBUF (no PE transpose);
    the host transposes back to [t, d] when unsharding.
"""

import sys

if "/opt/trn_rl_repo" not in sys.path:
    sys.path.insert(0, "/opt/trn_rl_repo")

from contextlib import ExitStack

import numpy as np

import concourse.bass as bass  # noqa: F401
import concourse.tile as tile
from concourse import bacc, mybir
from concourse.bass_utils import run_bass_kernel_spmd

N_CORES = 8
B, T, D, P = 8, 4096, 1024, 128
PANEL = 512                  # time-panel width (one PSUM bank of fp32)
N_PANELS = T // PANEL        # 8
EB = D // P                  # 8 output-channel blocks
DB = D // P                  # 8 contraction blocks

F32 = mybir.dt.float32
BF16 = mybir.dt.bfloat16
FP8 = mybir.dt.float8e4

# "bf16": both projections bf16 (rel err ~1.9e-3)
# "fp8dec": decay projection fp8-e4m3 DoubleRow (2x PE rate), injection
#           bf16 (rel err ~1.30e-2, measured bit-exact in numpy emulation
#           of the host-side quantization; gate is 2e-2)
MODE = "fp8dec"
W8_SCALE = 16.0   # W_decay^T scaled into fp8 normal range
X8_SCALE = 16.0   # x^T scaled to keep fp8 subnormals rare

_cached_nc = {}


def _build(repeat: int = 1, mode: str = MODE):
    key = (repeat, mode)
    if key in _cached_nc:
        return _cached_nc[key]

    nc = bacc.Bacc(
        "TRN2",
        target_bir_lowering=False,
        debug=False,
        enable_asserts=True,
        num_devices=N_CORES,
    )

    xt_ap = nc.dram_tensor("xt", [D, T], BF16, kind="ExternalInput").ap()
    if mode == "fp8dec":
        x8_ap = nc.dram_tensor("x8", [D, T], FP8, kind="ExternalInput").ap()
        wd8_ap = nc.dram_tensor("wd8", [D, D], FP8, kind="ExternalInput").ap()
    else:
        wdt_ap = nc.dram_tensor("wdt", [D, D], BF16, kind="ExternalInput").ap()
    wit_ap = nc.dram_tensor("wit", [D, D], BF16, kind="ExternalInput").ap()
    bd_ap = nc.dram_tensor("bd", [D], F32, kind="ExternalInput").ap()
    bi_ap = nc.dram_tensor("bi", [D], F32, kind="ExternalInput").ap()
    yt_ap = nc.dram_tensor("yt", [D, T], F32, kind="ExternalOutput").ap()

    with tile.TileContext(nc) as tc, ExitStack() as ctx:
        singles = ctx.enter_context(tc.tile_pool(name="singles", bufs=1))
        w_pool = ctx.enter_context(tc.tile_pool(name="w", bufs=1))
        x_pool = ctx.enter_context(tc.tile_pool(name="x", bufs=3))
        di_pool = ctx.enter_context(tc.tile_pool(name="di", bufs=3))
        st_pool = ctx.enter_context(tc.tile_pool(name="st", bufs=2))
        psum = ctx.enter_context(tc.tile_pool(name="psum", bufs=4, space="PSUM"))

        def load_panel(p):
            """Issue the x^T tile DMAs for panel p.

            bf16 tiles [128, 512] on the gpsimd queue; in fp8dec mode also
            4 fp8 DoubleRow tiles [128, 2, 512] (d = kj*256 + i*128 + p)
            on the vector queue."""
            ts = slice(p * PANEL, (p + 1) * PANEL)
            tiles = []
            for db in range(DB):
                t_ = x_pool.tile([P, PANEL], BF16, tag=f"xt{db}")
                nc.gpsimd.dma_start(t_[:], xt_ap[db * P:(db + 1) * P, ts])
                tiles.append(t_)
            t8 = []
            if mode == "fp8dec":
                for kj in range(DB // 2):
                    t_ = x_pool.tile([P, 2, PANEL], FP8, tag=f"x8{kj}")
                    nc.scalar.dma_start(
                        t_[:],
                        x8_ap[kj * 2 * P:(kj + 1) * 2 * P, ts].rearrange(
                            "(i p) t -> p i t", i=2
                        ),
                    )
                    t8.append(t_)
            return tiles, t8

        # prologue: weights interleaved with the first two x panels so the
        # first matmul group is unblocked after ~2 small DMAs
        wdT, wiT, wd8T = [], [], []
        xq = [load_panel(0)]
        if mode == "fp8dec":
            for kj in range(DB // 2):
                w_ = w_pool.tile([P, 2, D], FP8, tag=f"wd8{kj}")
                nc.sync.dma_start(
                    w_[:],
                    wd8_ap[kj * 2 * P:(kj + 1) * 2 * P, :].rearrange(
                        "(i p) e -> p i e", i=2
                    ),
                )
                wd8T.append(w_)
        else:
            for db in range(DB):
                w_ = w_pool.tile([P, D], BF16, tag=f"wd{db}")
                nc.sync.dma_start(w_[:], wdt_ap[db * P:(db + 1) * P, :])
                wdT.append(w_)
        xq.append(load_panel(1))
        for db in range(DB):
            w_ = w_pool.tile([P, D], BF16, tag=f"wi{db}")
            nc.sync.dma_start(w_[:], wit_ap[db * P:(db + 1) * P, :])
            wiT.append(w_)

        bd_sb = singles.tile([P, EB], F32, tag="bd")
        nc.sync.dma_start(bd_sb[:], bd_ap.rearrange("(f p) -> p f", p=P))
        bi_sb = singles.tile([P, EB], F32, tag="bi")
        nc.sync.dma_start(bi_sb[:], bi_ap.rearrange("(f p) -> p f", p=P))

        prev_st = [None] * EB
        total = repeat * N_PANELS
        dec_scale = (
            1.0 / (W8_SCALE * X8_SCALE) if mode == "fp8dec" else 1.0
        )
        for p_rep in range(total):
            p = p_rep % N_PANELS
            xt, x8t = xq.pop(0)
            if p_rep + 2 < total:
                xq.append(load_panel((p_rep + 2) % N_PANELS))

            for eb in range(EB):
                ebs = slice(eb * P, (eb + 1) * P)
                pzd = psum.tile([P, PANEL], F32, tag="pzd")
                if mode == "fp8dec":
                    for kj in range(DB // 2):
                        nc.tensor.matmul(
                            pzd[:], wd8T[kj][:, :, ebs], x8t[kj][:],
                            start=(kj == 0), stop=(kj == DB // 2 - 1),
                            perf_mode=mybir.MatmulPerfMode.DoubleRow,
                        )
                else:
                    for db in range(DB):
                        nc.tensor.matmul(
                            pzd[:], wdT[db][:, ebs], xt[db][:],
                            start=(db == 0), stop=(db == DB - 1),
                        )
                pzi = psum.tile([P, PANEL], F32, tag="pzi")
                for db in range(DB):
                    nc.tensor.matmul(
                        pzi[:], wiT[db][:, ebs], xt[db][:],
                        start=(db == 0), stop=(db == DB - 1),
                    )

                dec = di_pool.tile([P, PANEL], F32, tag="dec")
                nc.scalar.activation(
                    dec[:], pzd[:],
                    mybir.ActivationFunctionType.Sigmoid,
                    bias=bd_sb[:, eb:eb + 1], scale=dec_scale,
                )
                inj = di_pool.tile([P, PANEL], F32, tag="inj")
                nc.scalar.activation(
                    inj[:], pzi[:],
                    mybir.ActivationFunctionType.Identity,
                    bias=bi_sb[:, eb:eb + 1], scale=1.0,
                )

                st = st_pool.tile([P, PANEL], F32, tag=f"st{eb}")
                init = 0.0 if p_rep == 0 else prev_st[eb][:, PANEL - 1:PANEL]
                nc.vector.tensor_tensor_scan(
                    st[:], dec[:], inj[:], init,
                    mybir.AluOpType.mult, mybir.AluOpType.add,
                )
                prev_st[eb] = st

                nc.sync.dma_start(
                    yt_ap[eb * P:(eb + 1) * P, p * PANEL:(p + 1) * PANEL],
                    st[:],
                )

    nc.compile()
    _cached_nc[key] = nc
    return nc


def run(inputs: dict, trace: bool = False):
    """Run on 8 cores; returns (output [8, T, D], BassKernelResults)."""
    nc = _build()
    np_bf16 = mybir.dt.np(BF16)
    np_fp8 = mybir.dt.np(FP8)
    x = np.asarray(inputs["x_seq"], dtype=np.float32)
    wit = np.asarray(inputs["W_input"], dtype=np.float32).T.astype(np_bf16)
    bd = np.ascontiguousarray(np.asarray(inputs["b_decay"], dtype=np.float32))
    bi = np.ascontiguousarray(np.asarray(inputs["b_input"], dtype=np.float32))
    wd_t = np.asarray(inputs["W_decay"], dtype=np.float32).T
    if MODE == "fp8dec":
        wd8 = (wd_t * W8_SCALE).clip(-240, 240).astype(np_fp8)
        in_maps = [
            {
                "xt": x[b].T.astype(np_bf16),
                "x8": (x[b].T * X8_SCALE).clip(-240, 240).astype(np_fp8),
                "wd8": wd8,
                "wit": wit,
                "bd": bd,
                "bi": bi,
            }
            for b in range(N_CORES)
        ]
    else:
        wdt = wd_t.astype(np_bf16)
        in_maps = [
            {
                "xt": x[b].T.astype(np_bf16),
                "wdt": wdt,
                "wit": wit,
                "bd": bd,
                "bi": bi,
            }
            for b in range(N_CORES)
        ]
    res = run_bass_kernel_spmd(
        nc, in_maps, core_ids=list(range(N_CORES)), trace=trace
    )
    out = np.stack(
        [np.asarray(res.results[b]["yt"]).T for b in range(N_CORES)], axis=0
    )
    return np.ascontiguousarray(out), res


def kernel(x_seq, W_decay, b_decay, W_input, b_input) -> np.ndarray:
    out, _ = run(
        {
            "x_seq": x_seq,
            "W_decay": W_decay,
            "b_decay": b_decay,
            "W_input": W_input,
            "b_input": b_input,
        }
    )
    return out
